# revision 1
# baseline (speedup 1.0000x reference)
"""Cross-attention kernel for Trainium2, 8-core SPMD.

Problem (all fp32):
  x [2, 2048, 1024]; wq/wk/wv/w_proj [1024, 1024]; b_proj [1024]
  q = x[:, :1024] @ wq.T   (16 heads x 64)
  k, v = x @ wk.T, x @ wv.T
  out = softmax(q k^T / 8) v  -> proj + bias  -> [2, 1024, 1024]

Sharding: 8 cores = 2 (batch) x 4 (head-groups of 4 heads). Each core
computes its batch's QKV for its 4 heads, full attention for those heads,
and a partial projection (its 256 contraction rows of w_proj). Host sums
the 4 partials per batch and adds the bias (standard tensor-parallel
unshard).

Per-core layout ("T convention"): activations are kept feature-on-partition
(xT [c, n]); q/k are produced transposed (qT/kT [d, n]), v natural [n, d]
with an appended ones-column so the attn@v matmul also emits the softmax
denominator for free. The softmax max-subtraction is skipped (scores are
provably < ~10 for this problem, exp stays in fp32 range).

Schedule: inputs stream in chunk-interleaved across both DMA queue
families while q/k(pair0) and half the v-projection consume each x chunk
as it lands; scores(0) then runs with the rest of stage A interleaved as
PE filler (phased so every exp's SBUF slot is freed by earlier-emitted
work - the PE queue is strict FIFO and slot waits can otherwise
deadlock); attnv(h-1) interleaves per-j with scores(h) so the ACT
engine's exp stream (~73us floor) stays saturated; the projection tail
alternates evacuation engines and output DMA queues.
"""

import os
import numpy as np

import concourse.bacc as bacc
import concourse.bass as bass
import concourse.tile as tile
import concourse.mybir as mybir
from concourse.bass_utils import run_bass_kernel_spmd

F32 = mybir.dt.float32
# float32r: same fp32 bits, single-pass PE matmul (4x faster than fp32's
# two half-speed passes) at 11-bit-mantissa internal precision.
MM_DT = {
    "f32": mybir.dt.float32,
    "f32r": mybir.dt.float32r,
}[os.environ.get("KERNEL_MM_DT", "f32r")]

C = 1024          # model dim
N = 2048          # kv tokens
NQ = 1024         # query tokens
HPC = 4           # heads per core
D = 64            # head dim
DH = HPC * D      # per-core slice of C (256)
SCALE = D ** -0.5
P = 128

_CACHE: dict = {}


def _build():
    nc = bacc.Bacc("TRN2", target_bir_lowering=False, debug=False, num_devices=8)

    xT = nc.dram_tensor("xT", [C, N], MM_DT, kind="ExternalInput").ap()
    wqT = nc.dram_tensor("wqT", [C, DH], MM_DT, kind="ExternalInput").ap()
    wkT = nc.dram_tensor("wkT", [C, DH], MM_DT, kind="ExternalInput").ap()
    wvT = nc.dram_tensor("wvT", [C, DH], MM_DT, kind="ExternalInput").ap()
    wpT = nc.dram_tensor("wpT", [DH, C], MM_DT, kind="ExternalInput").ap()
    out = nc.dram_tensor("out", [NQ, C], F32, kind="ExternalOutput").ap()

    with tile.TileContext(nc) as tc, \
            nc.allow_low_precision(reason="fp32r matmul pipeline (fp32 bits, 11-bit mantissa in PE)"):
        _emit(tc, xT, wqT, wkT, wvT, wpT, out)

    nc.compile()
    return nc


def _emit(tc, xT, wqT, wkT, wvT, wpT, out):
    nc = tc.nc
    mm = nc.tensor.matmul
    Exp = mybir.ActivationFunctionType.Exp

    from contextlib import ExitStack

    with ExitStack() as ctx:
        # One shared slot class for every [128, 2048]-f32-sized tile: the 8
        # xT chunks + 3 QKV weights live through stage A, then those slots
        # recycle as exp(scores) tiles during attention.
        big = ctx.enter_context(tc.tile_pool(name="big", bufs=15))
        singles = ctx.enter_context(tc.tile_pool(name="singles", bufs=1))
        rcp = ctx.enter_context(tc.tile_pool(name="rcp", bufs=1))
        bcp = ctx.enter_context(tc.tile_pool(name="bcp", bufs=1))
        outp = ctx.enter_context(tc.tile_pool(name="outp", bufs=4))
        ps_big = ctx.enter_context(tc.tile_pool(name="ps_big", bufs=3, space="PSUM"))
        ps_sm = ctx.enter_context(tc.tile_pool(name="ps_sm", bufs=2, space="PSUM"))

        # ---- loads (per-chunk weight DMAs so the first matmul starts after
        # ~256KB of traffic instead of ~2MB; in first-use order)
        def load_w(name, dram):
            t = big.tile([P, 8, DH], MM_DT, name=name, tag="big")
            src = dram.rearrange("(a p) d -> p a d", p=P)
            for ci in range(8):
                nc.sync.dma_start(out=t[:, ci, :], in_=src[:, ci, :])
            return t

        wq_src = wqT.rearrange("(a p) d -> p a d", p=P)
        wk_src = wkT.rearrange("(a p) d -> p a d", p=P)
        wq_sb = big.tile([P, 8, DH], MM_DT, name="wq_sb", tag="big")
        wk_sb = big.tile([P, 8, DH], MM_DT, name="wk_sb", tag="big")
        xt = []
        for ci in range(8):
            t = big.tile([P, N], MM_DT, name=f"xt{ci}", tag="big")
            xt.append(t)
        # Two DMA queue families run concurrently: HWDGE (nc.sync) carries
        # wq + even x chunks, SWDGE (nc.gpsimd) carries wk + odd x chunks,
        # interleaved so chunk ci's inputs land just before its matmuls.
        wv_sb = big.tile([P, 8, DH], MM_DT, name="wv_sb", tag="big")
        wv_src = wvT.rearrange("(a p) d -> p a d", p=P)
        nc.sync.dma_start(out=wq_sb[:, 0, :], in_=wq_src[:, 0, :])
        nc.gpsimd.dma_start(out=wk_sb[:, 0, :], in_=wk_src[:, 0, :])
        nc.sync.dma_start(out=xt[0], in_=xT[0:P, :])
        nc.gpsimd.dma_start(out=xt[1], in_=xT[P:2 * P, :])
        for ci in range(1, 4):
            nc.sync.dma_start(out=wq_sb[:, ci, :], in_=wq_src[:, ci, :])
            nc.gpsimd.dma_start(out=wk_sb[:, ci, :], in_=wk_src[:, ci, :])
        nc.sync.dma_start(out=wv_sb[:, 0, :], in_=wv_src[:, 0, :])
        nc.gpsimd.dma_start(out=wv_sb[:, 1, :], in_=wv_src[:, 1, :])
        nc.sync.dma_start(out=xt[2], in_=xT[2 * P:3 * P, :])
        nc.gpsimd.dma_start(out=xt[3], in_=xT[3 * P:4 * P, :])
        for ci in range(2, 4):
            eng = nc.sync if ci % 2 == 0 else nc.gpsimd
            eng.dma_start(out=wv_sb[:, ci, :], in_=wv_src[:, ci, :])
        for ci in range(4, 6):
            nc.sync.dma_start(out=wq_sb[:, ci, :], in_=wq_src[:, ci, :])
            nc.gpsimd.dma_start(out=wk_sb[:, ci, :], in_=wk_src[:, ci, :])
        nc.sync.dma_start(out=xt[4], in_=xT[4 * P:5 * P, :])
        nc.gpsimd.dma_start(out=xt[5], in_=xT[5 * P:6 * P, :])
        for ci in range(6, 8):
            nc.sync.dma_start(out=wq_sb[:, ci, :], in_=wq_src[:, ci, :])
            nc.gpsimd.dma_start(out=wk_sb[:, ci, :], in_=wk_src[:, ci, :])
        for ci in range(4, 8):
            eng = nc.sync if ci % 2 == 0 else nc.gpsimd
            eng.dma_start(out=wv_sb[:, ci, :], in_=wv_src[:, ci, :])
        nc.sync.dma_start(out=xt[6], in_=xT[6 * P:7 * P, :])
        nc.gpsimd.dma_start(out=xt[7], in_=xT[7 * P:8 * P, :])


        ones_sb = singles.tile([P, D], MM_DT, name="ones", tag="ones")
        nc.vector.memset(ones_sb.bitcast(F32), 1.0)

        # Pre-trigger the ~2.7us exp table load while DMAs stream.
        dm = singles.tile([1, 1], MM_DT, name="dm", tag="dm")
        nc.scalar.activation(out=dm, in_=ones_sb[0:1, 0:1], func=Exp, scale=1.0)

        # ---- stage A: q/k/v projection emitters --------------------------
        qt = [singles.tile([P, NQ], MM_DT, name=f"qt{p}", tag=f"qt{p}") for p in range(2)]
        kt = [singles.tile([P, N], MM_DT, name=f"kt{p}", tag=f"kt{p}") for p in range(2)]
        v_sb = []
        for j in range(16):
            t = singles.tile([P, HPC, D + 1], MM_DT, name=f"v{j}", tag=f"v{j}")
            v_sb.append(t)

        def q_proj_gen(pair):
            ps = ps_big.tile([P, 1024], F32, name=f"ps_q{pair}", tag="psb")
            for ci in range(8):
                lw = wq_sb[:, ci, pair * P:(pair + 1) * P]
                for nh in range(2):
                    mm(ps[:, nh * 512:(nh + 1) * 512], lw,
                       xt[ci][:, nh * 512:(nh + 1) * 512],
                       start=(ci == 0), stop=(ci == 7), skip_group_check=True)
                yield
            nc.vector.tensor_copy(qt[pair], ps)
            yield

        def k_proj_gen(pair, half):
            ps = ps_big.tile([P, 1024], F32, name=f"ps_k{pair}_{half}", tag="psb")
            for ci in range(8):
                lw = wk_sb[:, ci, pair * P:(pair + 1) * P]
                for nh in range(2):
                    nk0 = half * 1024 + nh * 512
                    mm(ps[:, nh * 512:(nh + 1) * 512], lw,
                       xt[ci][:, nk0:nk0 + 512],
                       start=(ci == 0), stop=(ci == 7), skip_group_check=True)
                yield
            nc.vector.tensor_copy(kt[pair][:, half * 1024:(half + 1) * 1024], ps)
            yield

        def v_group_gen(j):
            # v pass 2 (ci 4..7), accumulated onto pass 1's partial in SBUF
            ps = ps_sm.tile([P, 512], F32, name=f"ps_v2_{j}", tag="pss")
            for ci in range(4, 8):
                mm(ps[:, 0:DH], xt[ci][:, j * P:(j + 1) * P],
                   wv_sb[:, ci, :],
                   start=(ci == 4), stop=(ci == 7), skip_group_check=True)
                yield
            nc.vector.tensor_add(
                v_sb[j][:, :, 0:D], v_sb[j][:, :, 0:D],
                ps[:, 0:DH].rearrange("p (h d) -> p h d", h=HPC))
            yield

        # ---- attention helpers -------------------------------------------
        out_h = [singles.tile([D, NQ], MM_DT, name=f"oh{h}", tag=f"oh{h}")
                 for h in range(HPC)]

        def alloc_ets(h):
            return [big.tile([P, 2, NQ], MM_DT, name=f"et{h}_{k}", tag="big")
                    for k in range(8)]

        def scores_j(h, ets, j):
            pair, po = h // 2, 64 * (h % 2)
            ps = ps_big.tile([P, 1024], F32, name=f"ps_s{h}_{j}", tag="psb")
            lw = kt[pair][po:po + 64, j * P:(j + 1) * P]
            for nh in range(2):
                mm(ps[:, nh * 512:(nh + 1) * 512], lw,
                   qt[pair][po:po + 64, nh * 512:(nh + 1) * 512],
                   start=True, stop=True)
            nc.scalar.activation(out=ets[j // 2][:, j % 2, :], in_=ps,
                                 func=Exp, scale=SCALE)

        def attnv_j(h, ets, ps_o, j):
            lw = v_sb[j][:, h, :]               # [128, 65] (col 64 = ones)
            for nh in range(2):
                mm(ps_o[0:D + 1, nh * 512:(nh + 1) * 512], lw,
                   ets[j // 2][:, j % 2, nh * 512:(nh + 1) * 512],
                   start=(j == 0), stop=(j == 15), skip_group_check=True)

        def norm(h, ps_o):
            # rows 0..63 = unnormalized out^T, row 64 = sum(exp) denominator
            rc = rcp.tile([D + 1, NQ], MM_DT, name=f"rc{h}", tag="rc")
            nc.vector.reciprocal(rc[D:D + 1, :], ps_o[D:D + 1, :])
            # broadcast 1/denom across partitions via ones-outer-product
            bc = bcp.tile([D, NQ], MM_DT, name=f"bc{h}", tag="bc")
            for nh in range(2):
                pb = ps_sm.tile([P, 512], F32, name=f"ps_b{h}_{nh}", tag="pss")
                mm(pb[0:D, :], ones_sb[D:D + 1, 0:D],
                   rc[D:D + 1, nh * 512:(nh + 1) * 512],
                   start=True, stop=True)
                nc.vector.tensor_copy(bc[:, nh * 512:(nh + 1) * 512], pb[0:D, :])
            nc.vector.tensor_mul(out_h[h], ps_o[0:D, :], bc)

        # ---- A1: q/k for head-pair 0, ci-outer so each arriving xT chunk
        # is consumed immediately (3 psum groups accumulate in parallel) ---
        ps_qa = ps_big.tile([P, 1024], F32, name="ps_q0", tag="psb")
        ps_ka = [ps_big.tile([P, 1024], F32, name=f"ps_k0_{half}", tag="psb")
                 for half in range(2)]

        def a1_part(cis):
            for ci in cis:
                lw = wq_sb[:, ci, 0:P]
                for nh in range(2):
                    mm(ps_qa[:, nh * 512:(nh + 1) * 512], lw,
                       xt[ci][:, nh * 512:(nh + 1) * 512],
                       start=(ci == 0), stop=(ci == 7), skip_group_check=True)
                lw = wk_sb[:, ci, 0:P]
                for half in range(2):
                    for nh in range(2):
                        nk0 = half * 1024 + nh * 512
                        mm(ps_ka[half][:, nh * 512:(nh + 1) * 512], lw,
                           xt[ci][:, nk0:nk0 + 512],
                           start=(ci == 0), stop=(ci == 7), skip_group_check=True)

        # v passes 1a/1b (ci 0,1 then 2,3) are placed exactly at the two
        # input-arrival waits (xt2/3 and xt4/5); pass 2 finishes in B0.
        a1_part(range(2))
        for j in range(16):
            ps = ps_sm.tile([P, 512], F32, name=f"ps_v1a_{j}", tag="pss")
            for ci in range(2):
                mm(ps[:, 0:DH], xt[ci][:, j * P:(j + 1) * P],
                   wv_sb[:, ci, :],
                   start=(ci == 0), stop=(ci == 1), skip_group_check=True)
            nc.vector.tensor_copy(
                v_sb[j][:, :, 0:D],
                ps[:, 0:DH].rearrange("p (h d) -> p h d", h=HPC))
            nc.gpsimd.memset(v_sb[j][:, :, D:D + 1].bitcast(F32), 1.0)
        a1_part(range(2, 4))
        for j in range(16):
            ps = ps_sm.tile([P, 512], F32, name=f"ps_v1b_{j}", tag="pss")
            for ci in range(2, 4):
                mm(ps[:, 0:DH], xt[ci][:, j * P:(j + 1) * P],
                   wv_sb[:, ci, :],
                   start=(ci == 2), stop=(ci == 3), skip_group_check=True)
            nc.vector.tensor_add(
                v_sb[j][:, :, 0:D], v_sb[j][:, :, 0:D],
                ps[:, 0:DH].rearrange("p (h d) -> p h d", h=HPC))
        a1_part(range(4, 8))
        nc.vector.tensor_copy(qt[0], ps_qa)
        for half in range(2):
            nc.vector.tensor_copy(kt[0][:, half * 1024:(half + 1) * 1024],
                                  ps_ka[half])

        # ---- B0: scores(0) with the rest of stage A as PE filler ---------
        # PE is strict FIFO, so each scores_j may only be emitted after the
        # filler whose completion frees the SBUF slot its exp needs:
        # 4 slots are free at B0 start (exps j0..7), +1 after q(1) (j8,9),
        # +1 after k(1,1) (j10,11), and the rest only after v releases the
        # xT chunks (j12..15 come last).
        from itertools import chain

        def pull(gen, k):
            for _ in range(k):
                if next(gen, None) is None:
                    return False
            return True

        ets_prev = alloc_ets(0)
        f1 = q_proj_gen(1)                                   # 17 units
        for j in range(6):
            scores_j(0, ets_prev, j)
            pull(f1, 3)
        for _ in f1:
            pass
        f2 = chain(k_proj_gen(1, 0), k_proj_gen(1, 1))       # 34 units
        for j in range(6, 10):
            scores_j(0, ets_prev, j)
            pull(f2, 9)
        for _ in f2:
            pass
        f3 = chain(*(v_group_gen(j) for j in range(16)))     # 80 units
        for j in range(10, 12):
            scores_j(0, ets_prev, j)
            pull(f3, 12)
        for _ in f3:
            pass
        for j in range(12, 16):
            scores_j(0, ets_prev, j)

        # ---- pipelined attention: attnv(h-1) interleaved with scores(h) --
        ps_o_prev = ps_big.tile([P, 1024], F32, name="ps_o0", tag="psb")
        for h in range(1, HPC):
            ets_h = alloc_ets(h)
            ps_o_h = None
            for j in range(16):
                scores_j(h, ets_h, j)
                attnv_j(h - 1, ets_prev, ps_o_prev, j)
            norm(h - 1, ps_o_prev)
            ets_prev = ets_h
            ps_o_prev = ps_big.tile([P, 1024], F32, name=f"ps_o{h}", tag="psb")
        wp_h = []
        for h in range(HPC):
            t = big.tile([D, C], MM_DT, name=f"wp{h}", tag="big")
            nc.sync.dma_start(out=t, in_=wpT[h * D:(h + 1) * D, :])
            wp_h.append(t)
        for j in range(16):
            attnv_j(HPC - 1, ets_prev, ps_o_prev, j)
        norm(HPC - 1, ps_o_prev)

        # ---- partial projection ------------------------------------------
        for m in range(8):
            ps = ps_big.tile([P, 1024], F32, name=f"ps_f{m}", tag="psb")
            for h in range(HPC):
                lw = out_h[h][:, m * P:(m + 1) * P]   # [64, 128]
                for nh in range(2):
                    mm(ps[:, nh * 512:(nh + 1) * 512], lw,
                       wp_h[h][:, nh * 512:(nh + 1) * 512],
                       start=(h == 0), stop=(h == HPC - 1), skip_group_check=True)
            fin = outp.tile([P, 1024], F32, name=f"fin{m}", tag="fin")
            nc.scalar.copy(fin[:, 0:512], ps[:, 0:512])
            nc.vector.tensor_copy(fin[:, 512:1024], ps[:, 512:1024])
            nc.sync.dma_start(out=out[m * P:(m + 1) * P, 0:512],
                              in_=fin[:, 0:512])
            nc.gpsimd.dma_start(out=out[m * P:(m + 1) * P, 512:1024],
                                in_=fin[:, 512:1024])


def _get_nc():
    if "nc" not in _CACHE:
        _CACHE["nc"] = _build()
    return _CACHE["nc"]


def kernel(x, wq, wk, wv, w_proj, b_proj):
    x = np.asarray(x, dtype=np.float32)
    wq = np.asarray(wq, dtype=np.float32)
    wk = np.asarray(wk, dtype=np.float32)
    wv = np.asarray(wv, dtype=np.float32)
    w_proj = np.asarray(w_proj, dtype=np.float32)
    b_proj = np.asarray(b_proj, dtype=np.float32)

    nc = _get_nc()
    in_maps = []
    for core in range(8):
        b, g = divmod(core, 4)
        sl = slice(g * DH, (g + 1) * DH)
        in_maps.append({
            "xT": np.ascontiguousarray(x[b].T),
            "wqT": np.ascontiguousarray(wq[sl, :].T),
            "wkT": np.ascontiguousarray(wk[sl, :].T),
            "wvT": np.ascontiguousarray(wv[sl, :].T),
            "wpT": np.ascontiguousarray(w_proj[:, sl].T),
        })

    res = run_bass_kernel_spmd(nc, in_maps, core_ids=list(range(8)),
                               trace=bool(int(os.environ.get("KERNEL_TRACE", "0"))))
    _CACHE["last_results"] = res
    outs = [res.results[c]["out"] for c in range(8)]
    full = np.stack([outs[0] + outs[1] + outs[2] + outs[3],
                     outs[4] + outs[5] + outs[6] + outs[7]])
    full += b_proj[None, None, :]
    return full.astype(np.float32)



# revision 29
# speedup vs baseline: 1.3177x; 1.3177x over previous
"""Cross-attention kernel for Trainium2, 8-core SPMD.

Problem (all fp32):
  x [2, 2048, 1024]; wq/wk/wv/w_proj [1024, 1024]; b_proj [1024]
  q = x[:, :1024] @ wq.T   (16 heads x 64)
  k, v = x @ wk.T, x @ wv.T
  out = softmax(q k^T / 8) v  -> proj + bias  -> [2, 1024, 1024]

Sharding: 8 cores = 2 (batch) x 4 (head-groups of 4 heads). Each core
computes its batch's QKV for its 4 heads, full attention for those heads,
and a partial projection (its 256 contraction rows of w_proj). Host sums
the 4 partials per batch and adds the bias.

v2 design notes (PE-row economy; one "row" = one moving-operand free
element = 1 PE cycle at full clock):
  - Everything on-chip is bf16 except PSUM accumulation, the f32 v
    partials, and the final output. Softmax here is flat (scores ~N(0,1)
    over 2048 keys) so quantization error averages out far below the
    2e-2 gate. bf16 also halves input DMA, pulling the exp stream in.
  - attn@v runs exp-STATIONARY: lhsT = exp-scores chunk [128 kv, 128 q],
    moving = v+ones [128, 65]. Output [128 q, 65] uses all 128 psum
    partitions (vs 65 when v is stationary): 65536 -> 33280 rows. The
    ones column makes column 64 the softmax denominator.
  - Normalization is a DVE reciprocal + per-partition tensor_scalar
    multiply straight out of PSUM (no broadcast matmuls, no extra copy).
  - The projection contracts head PAIRS (K=128): normalized attn-out is
    packed per pair [q, 128], DMA-XBAR-transposed to [128 dh, q], so the
    projection needs half the accumulation steps: 32768 -> 16384 rows.
  - Loads ride one HWDGE queue in exact consumption order (w chunk
    before its x chunk); q/k for head-pair 0 consume each x chunk on
    arrival, v (2 chunk-passes) and q/k pair 1 fill the PE while the
    ACT exp stream (the second bottleneck, ~64us) runs.
"""

import os
import numpy as np
import ml_dtypes

import concourse.bacc as bacc
import concourse.bass as bass
import concourse.tile as tile
import concourse.mybir as mybir
from concourse.bass_utils import run_bass_kernel_spmd

F32 = mybir.dt.float32
BF16 = mybir.dt.bfloat16

C = 1024          # model dim
N = 2048          # kv tokens
NQ = 1024         # query tokens
HPC = 4           # heads per core
D = 64            # head dim
DH = HPC * D      # per-core slice of C (256)
SCALE = D ** -0.5
P = 128

_CACHE: dict = {}


def _build():
    nc = bacc.Bacc("TRN2", target_bir_lowering=False, debug=False, num_devices=8)

    xT = nc.dram_tensor("xT", [C, N], BF16, kind="ExternalInput").ap()
    # wA = [wq(pair0 128) | wk(pair0 128) | wv(256)], wB = [wq(pair1) | wk(pair1)]
    wA = nc.dram_tensor("wA", [C, 512], BF16, kind="ExternalInput").ap()
    wB = nc.dram_tensor("wB", [C, 256], BF16, kind="ExternalInput").ap()
    wpT = nc.dram_tensor("wpT", [DH, C], BF16, kind="ExternalInput").ap()
    # partial per-core projection; host sums the 4 head-group partials in
    # f32, so bf16 partials only cost ~0.4% relative - well inside the gate
    out = nc.dram_tensor("out", [NQ, C], BF16, kind="ExternalOutput").ap()

    with tile.TileContext(nc) as tc, \
            nc.allow_low_precision(reason="bf16 attention pipeline, f32 psum accumulation"):
        _emit(tc, xT, wA, wB, wpT, out)

    nc.compile()
    return nc


def _emit(tc, xT, wA, wB, wpT, out):
    nc = tc.nc
    mm = nc.tensor.matmul
    Exp = mybir.ActivationFunctionType.Exp
    MUL = mybir.AluOpType.mult

    from contextlib import ExitStack
    from itertools import chain

    with ExitStack() as ctx:
        sb = ctx.enter_context(tc.tile_pool(name="sb", bufs=1))
        rcp = ctx.enter_context(tc.tile_pool(name="rcp", bufs=2))
        finp = ctx.enter_context(tc.tile_pool(name="finp", bufs=4))
        pA = ctx.enter_context(tc.tile_pool(name="pA", bufs=3, space="PSUM"))
        pS = ctx.enter_context(tc.tile_pool(name="pS", bufs=2, space="PSUM"))

        def til(shape, dt, name):
            return sb.tile(shape, dt, name=name, tag=name)

        # ---- tiles ----
        wA_sb = til([P, 8, 512], BF16, "wA_sb")
        wB_sb = til([P, 8, 256], BF16, "wB_sb")
        wp_sb = til([P, 2, C], BF16, "wp_sb")
        xa = [til([P, NQ], BF16, f"xa{ci}") for ci in range(8)]
        xb = [til([P, NQ], BF16, f"xb{ci}") for ci in range(8)]
        qt = [til([P, NQ], BF16, f"qt{p}") for p in range(2)]
        kt = [[til([P, NQ], BF16, f"kt{p}_{hf}") for hf in range(2)] for p in range(2)]
        vb = [til([P, HPC, D + 1], BF16, f"vb{j}") for j in range(16)]
        # triple-buffered so attnv(h) may lag up to two heads behind exp(h)
        ets = [[til([P, NQ], BF16, f"ets{s}_{j}") for j in range(16)] for s in range(3)]
        ap_ = [til([P, 8, P], BF16, f"ap{p}") for p in range(2)]
        aT = [til([P, 8, P], BF16, f"aT{p}") for p in range(2)]
        ones = til([1, 1], BF16, "ones")
        onesD = til([1, D], BF16, "onesD")
        bc = til([D, NQ], BF16, "bc")

        # Pre-trigger the exp table load while DMAs stream.
        nc.vector.memset(ones, 1.0)
        nc.vector.memset(onesD, 1.0)
        dm = til([1, 1], BF16, "dm")
        nc.scalar.activation(out=dm, in_=ones, func=Exp, scale=1.0)

        # ---- loads: one HWDGE queue, exact consumption order ----
        wA_src = wA.rearrange("(a p) d -> p a d", p=P)
        for ci in range(8):
            nc.sync.dma_start(out=wA_sb[:, ci, :], in_=wA_src[:, ci, :])
            nc.sync.dma_start(out=xa[ci], in_=xT[ci * P:(ci + 1) * P, 0:NQ])
        for ci in range(8):
            nc.sync.dma_start(out=xb[ci], in_=xT[ci * P:(ci + 1) * P, NQ:N])
        nc.sync.dma_start(out=wB_sb, in_=wB.rearrange("(a p) d -> p a d", p=P))
        nc.sync.dma_start(out=wp_sb, in_=wpT.rearrange("(a p) d -> p a d", p=P))

        # ---- stage A: q/k pair-0 ci-outer + v pass 1 (ci 0..3), consuming
        # chunks on arrival (stage A is DMA-bound; the rest is B-phase filler)
        ps_q = pA.tile([P, NQ], F32, name="ps_q0", tag="pA")
        ps_k0 = pA.tile([P, NQ], F32, name="ps_k0_0", tag="pA")

        for ci in range(8):
            lw = wA_sb[:, ci, 0:P]
            for nh in range(2):
                mm(ps_q[:, nh * 512:(nh + 1) * 512], lw,
                   xa[ci][:, nh * 512:(nh + 1) * 512],
                   start=(ci == 0), stop=(ci == 7), skip_group_check=True)
            lw = wA_sb[:, ci, P:2 * P]
            for nh in range(2):
                mm(ps_k0[:, nh * 512:(nh + 1) * 512], lw,
                   xa[ci][:, nh * 512:(nh + 1) * 512],
                   start=(ci == 0), stop=(ci == 7), skip_group_check=True)
        # evacuations split across DVE and ACT (GPSIMD cannot read PSUM)
        nc.vector.tensor_copy(qt[0], ps_q)
        nc.scalar.copy(kt[0][0], ps_k0)

        k0h1_ps = []

        def k0h1_gen():
            ps = pA.tile([P, NQ], F32, name="ps_k0_1", tag="pA")
            k0h1_ps.append(ps)
            for ci in range(8):
                lw = wA_sb[:, ci, P:2 * P]
                for nh in range(2):
                    mm(ps[:, nh * 512:(nh + 1) * 512], lw,
                       xb[ci][:, nh * 512:(nh + 1) * 512],
                       start=(ci == 0), stop=(ci == 7), skip_group_check=True)
                yield

        # ---- B-phase helpers ----
        def scores_j(h, j):
            pair, po = h // 2, D * (h % 2)
            ps = pA.tile([P, NQ], F32, name=f"ps_s{h}_{j}", tag="pA")
            hf, jj = j // 8, j % 8
            lw = kt[pair][hf][po:po + D, jj * P:(jj + 1) * P]
            for nh in range(2):
                mm(ps[:, nh * 512:(nh + 1) * 512], lw,
                   qt[pair][po:po + D, nh * 512:(nh + 1) * 512],
                   start=True, stop=True)
            nc.scalar.activation(out=ets[h % 3][j], in_=ps, func=Exp, scale=SCALE)

        def attnv_norm_m(h, m):
            ps = pS.tile([P, 512], F32, name=f"ps_av{h}_{m}", tag="pS")
            for j in range(16):
                mm(ps[:, 0:D + 1], ets[h % 3][j][:, m * P:(m + 1) * P],
                   vb[j][:, h, :], start=(j == 0), stop=(j == 15),
                   skip_group_check=True)
            p, os_ = h // 2, D * (h % 2)
            rc = rcp.tile([P, 1], F32, name=f"rc{h}_{m}", tag="rc")
            nc.vector.reciprocal(rc, ps[:, D:D + 1])
            nc.vector.tensor_scalar(ap_[p][:, m, os_:os_ + D], ps[:, 0:D],
                                    rc[:, 0:1], None, MUL)

        def proj_m(m):
            ps = pA.tile([P, NQ], F32, name=f"ps_f{m}", tag="pA")
            for p in range(2):
                lw = aT[p][:, m, :]
                for nh in range(2):
                    mm(ps[:, nh * 512:(nh + 1) * 512], lw,
                       wp_sb[:, p, nh * 512:(nh + 1) * 512],
                       start=(p == 0), stop=(p == 1), skip_group_check=True)
            fin = finp.tile([P, NQ], BF16, name=f"fin{m}", tag="fin")
            nc.scalar.copy(fin[:, 0:512], ps[:, 0:512])
            nc.vector.tensor_copy(fin[:, 512:1024], ps[:, 512:1024])
            nc.sync.dma_start(out=out[m * P:(m + 1) * P, :], in_=fin)

        # ---- B-phase fillers: q/k pair-1, k pair-0 half-1 is already done;
        # v projection (all 8 chunks, one psum group per kv chunk)
        def q1_gen():
            ps = pA.tile([P, NQ], F32, name="ps_q1", tag="pA")
            for ci in range(8):
                lw = wB_sb[:, ci, 0:P]
                for nh in range(2):
                    mm(ps[:, nh * 512:(nh + 1) * 512], lw,
                       xa[ci][:, nh * 512:(nh + 1) * 512],
                       start=(ci == 0), stop=(ci == 7), skip_group_check=True)
                yield
            nc.vector.tensor_copy(qt[1], ps)
            yield

        def k1_gen(hf):
            ps = pA.tile([P, NQ], F32, name=f"ps_k1_{hf}", tag="pA")
            xh = xa if hf == 0 else xb
            for ci in range(8):
                lw = wB_sb[:, ci, P:2 * P]
                for nh in range(2):
                    mm(ps[:, nh * 512:(nh + 1) * 512], lw,
                       xh[ci][:, nh * 512:(nh + 1) * 512],
                       start=(ci == 0), stop=(ci == 7), skip_group_check=True)
                yield
            nc.vector.tensor_copy(kt[1][hf], ps)
            yield

        def v_gen():
            for j in range(16):
                ps = pS.tile([P, 512], F32, name=f"ps_v_{j}", tag="pS")
                nc.gpsimd.memset(vb[j][:, :, D:D + 1], 1.0)
                xh, jj = (xa, j) if j < 8 else (xb, j - 8)
                for ci in range(8):
                    mm(ps[:, 0:DH], xh[ci][:, jj * P:(jj + 1) * P], wA_sb[:, ci, 2 * P:],
                       start=(ci == 0), stop=(ci == 7), skip_group_check=True)
                    if ci % 4 == 3:
                        yield
                nc.vector.tensor_copy(
                    vb[j][:, :, 0:D],
                    ps[:, 0:DH].rearrange("p (h d) -> p h d", h=HPC))
                yield

        def pull(gen, k):
            for _ in range(k):
                if next(gen, None) is None:
                    return False
            return True

        fillers = chain(k0h1_gen(), q1_gen(), k1_gen(0), v_gen(), k1_gen(1))

        # ---- B0: scores/exp head 0, fillers keep PE busy between slot waits
        # (the kt[0][1] evacuation is emitted immediately before its first
        # consumer: the scheduler's wait-coalescing loses the RAW edge when
        # other PE work sits between them)
        for j in range(16):
            if j == 8:
                nc.vector.tensor_copy(kt[0][1], k0h1_ps[0])
            scores_j(0, j)
            pull(fillers, 3)

        # ---- B1: scores head 1; fillers drain (paced: the PE exec queue's
        # reorder window is ~32 deep, so big emission blobs jam it)
        for j in range(16):
            scores_j(1, j)
            pull(fillers, 3)
        for _ in fillers:
            pass

        # head-3 half of ap_[1] is never written (head 3 skips the packed
        # path); zero it so its transposes move defined bits
        nc.gpsimd.memset(ap_[1][:, :, D:P], 0.0)

        # ---- B2: scores head 2 | attnv heads 0+1 | transposes pair 0 ----
        for m in range(8):
            scores_j(2, 2 * m)
            scores_j(2, 2 * m + 1)
            attnv_norm_m(0, m)
            attnv_norm_m(1, m)
            eng = nc.sync if m % 2 == 0 else nc.scalar
            eng.dma_start_transpose(aT[0][:, m, :], ap_[0][:, m, :])

        # ---- head 3 runs v-STATIONARY (psum [65, q]): each exp chunk is
        # consumed as it is produced inside one open accumulation group, and
        # the output lands directly in [d, q] layout (aT rows) - no transpose
        # and no post-exp attnv stall for the final head. Head 2 stays on the
        # packed path; its transposes carry a zeroed head-3 half that the
        # head-3 normalization overwrites.
        ps_o3 = pA.tile([P, NQ], F32, name="ps_o3", tag="pA")

        def attnv_old3(j):
            lw = vb[j][:, 3, :]                  # [128, 65] (col 64 = ones)
            for nh in range(2):
                mm(ps_o3[0:D + 1, nh * 512:(nh + 1) * 512], lw,
                   ets[0][j][:, nh * 512:(nh + 1) * 512],
                   start=(j == 0), stop=(j == 15), skip_group_check=True)

        # ---- B3: scores head 3 | attnv head 3 chases its exp stream |
        # attnv head 2 + transposes pair 1 ----
        for m in range(8):
            scores_j(3, 2 * m)
            if m > 0:
                attnv_old3(2 * m - 2)
            scores_j(3, 2 * m + 1)
            if m > 0:
                attnv_old3(2 * m - 1)
            attnv_norm_m(2, m)
            eng = nc.sync if m % 2 == 0 else nc.scalar
            eng.dma_start_transpose(aT[1][:, m, :], ap_[1][:, m, :])
        attnv_old3(14)
        attnv_old3(15)

        # ---- B4 tail: normalize head 3 straight into aT rows, project ----
        # nh=0 chain completes first: proj(m<4) depends only on that half.
        # quarter-granular normalize chain: proj(0) only needs quarter 0,
        # so it starts ~2us after the last exp instead of ~3.5
        rc = rcp.tile([1, NQ], BF16, name="rco3", tag="rco")
        aTv = aT[1][D:2 * D, :, :].rearrange("p a b -> p (a b)")
        qsl = [slice(qq * 256, (qq + 1) * 256) for qq in range(4)]
        for qq in range(4):
            nc.vector.reciprocal(rc[0:1, qsl[qq]], ps_o3[D:D + 1, qsl[qq]])
        pbs = []
        for qq in range(0, 4, 2):
            pb = pS.tile([P, 512], F32, name=f"pb3_{qq}", tag="pS")
            mm(pb[0:D, 0:256], onesD, rc[0:1, qsl[qq]], start=True, stop=True)
            mm(pb[0:D, 256:512], onesD, rc[0:1, qsl[qq + 1]], start=True, stop=True)
            pbs.append(pb)
        for qq in range(4):
            nc.scalar.copy(bc[:, qsl[qq]], pbs[qq // 2][0:D, qsl[qq % 2]])
        for qq in range(4):
            nc.vector.tensor_mul(aTv[:, qsl[qq]], ps_o3[0:D, qsl[qq]], bc[:, qsl[qq]])
        for m in range(8):
            proj_m(m)


def _get_nc():
    if "nc" not in _CACHE:
        _CACHE["nc"] = _build()
    return _CACHE["nc"]


def kernel(x, wq, wk, wv, w_proj, b_proj):
    x = np.asarray(x, dtype=np.float32)
    wq = np.asarray(wq, dtype=np.float32)
    wk = np.asarray(wk, dtype=np.float32)
    wv = np.asarray(wv, dtype=np.float32)
    w_proj = np.asarray(w_proj, dtype=np.float32)
    b_proj = np.asarray(b_proj, dtype=np.float32)
    bf = ml_dtypes.bfloat16

    nc = _get_nc()
    in_maps = []
    for core in range(8):
        b, g = divmod(core, 4)
        sl = slice(g * DH, (g + 1) * DH)
        wqT = wq[sl, :].T
        wkT = wk[sl, :].T
        wvT = wv[sl, :].T
        in_maps.append({
            "xT": np.ascontiguousarray(x[b].T).astype(bf),
            "wA": np.ascontiguousarray(
                np.hstack([wqT[:, 0:P], wkT[:, 0:P], wvT])).astype(bf),
            "wB": np.ascontiguousarray(
                np.hstack([wqT[:, P:2 * P], wkT[:, P:2 * P]])).astype(bf),
            "wpT": np.ascontiguousarray(w_proj[:, sl].T).astype(bf),
        })

    res = run_bass_kernel_spmd(nc, in_maps, core_ids=list(range(8)),
                               trace=bool(int(os.environ.get("KERNEL_TRACE", "0"))))
    _CACHE["last_results"] = res
    outs = [np.asarray(res.results[c]["out"]).astype(np.float32) for c in range(8)]
    full = np.stack([outs[0] + outs[1] + outs[2] + outs[3],
                     outs[4] + outs[5] + outs[6] + outs[7]])
    full += b_proj[None, None, :]
    return full.astype(np.float32)


# revision 54
# speedup vs baseline: 1.3490x; 1.0237x over previous
"""Cross-attention kernel for Trainium2, 8-core SPMD.

Problem (all fp32):
  x [2, 2048, 1024]; wq/wk/wv/w_proj [1024, 1024]; b_proj [1024]
  q = x[:, :1024] @ wq.T   (16 heads x 64)
  k, v = x @ wk.T, x @ wv.T
  out = softmax(q k^T / 8) v  -> proj + bias  -> [2, 1024, 1024]

Sharding: 8 cores = 2 (batch) x 4 (head-groups of 4 heads). Each core
computes its batch's QKV for its 4 heads, full attention for those heads,
and a partial projection (its 256 contraction rows of w_proj). Host sums
the 4 partials per batch and adds the bias.

v2 design notes (PE-row economy: modeled matmul time = moving-operand
free size x cycles/row; K and M are free, so pack output partitions and
contraction per call):
  - Everything on-chip is bf16 except PSUM accumulation and the softmax
    reciprocals. Softmax here is flat (scores ~N(0,1) over 2048 keys) so
    quantization averages out far below the 2e-2 gate (measured 5.8e-3).
    bf16 halves input DMA; x loads split into query-token and kv-tail
    halves so the first exp fires after ~half the input bytes (t0~14us).
  - attn@v for heads 0-2 runs exp-STATIONARY: lhsT = exp-scores chunk
    [128 kv, 128 q], moving = v+ones [128, 65]; all 128 psum partitions
    carry output (vs 65 when v is stationary). The ones column makes
    column 64 the softmax denominator; normalization is a DVE reciprocal
    + per-partition tensor_scalar straight out of PSUM.
  - Head 3 (the last exp consumer) runs v-STATIONARY into one open psum
    group so each exp chunk is consumed as produced and the output lands
    pre-transposed in [d, q]; its normalization broadcasts 1/denom via a
    ones-outer-product matmul. This removes the post-exp-stream attnv
    stall and half the pair-1 transposes.
  - The projection contracts head PAIRS (K=128): packed attn-out
    [q, 128] tiles are DMA-XBAR-transposed to [128 dh, q], halving the
    projection's accumulation steps.
  - The ACT exp stream (~64us busy) is the window pacer; QKV-remainder
    work (k half-1, q/k pair-1, v) is generator-fed filler pulled
    between scores emissions, rate-matched to ~1 exp/us. Evacuations
    whose consumer is emitted far later are emitted ADJACENT to the
    first consumer: the scheduler's wait-coalescing can otherwise lose
    the RAW edge to a hoisted Ldweights (check_sync.py audits the
    compiled BIR for this).
"""

import os
import numpy as np
import ml_dtypes

import concourse.bacc as bacc
import concourse.bass as bass
import concourse.tile as tile
import concourse.mybir as mybir
from concourse.bass_utils import run_bass_kernel_spmd

F32 = mybir.dt.float32
BF16 = mybir.dt.bfloat16

C = 1024          # model dim
N = 2048          # kv tokens
NQ = 1024         # query tokens
HPC = 4           # heads per core
D = 64            # head dim
DH = HPC * D      # per-core slice of C (256)
SCALE = D ** -0.5
P = 128

_CACHE: dict = {}


def _build():
    nc = bacc.Bacc("TRN2", target_bir_lowering=False, debug=False, num_devices=8)

    xT = nc.dram_tensor("xT", [C, N], BF16, kind="ExternalInput").ap()
    # wA = [wq(pair0 128) | wk(pair0 128) | wv(256)], wB = [wq(pair1) | wk(pair1)]
    wA = nc.dram_tensor("wA", [C, 512], BF16, kind="ExternalInput").ap()
    wB = nc.dram_tensor("wB", [C, 256], BF16, kind="ExternalInput").ap()
    wpT = nc.dram_tensor("wpT", [DH, C], BF16, kind="ExternalInput").ap()
    # partial per-core projection; host sums the 4 head-group partials in
    # f32, so bf16 partials only cost ~0.4% relative - well inside the gate
    out = nc.dram_tensor("out", [NQ, C], BF16, kind="ExternalOutput").ap()

    with tile.TileContext(nc) as tc, \
            nc.allow_low_precision(reason="bf16 attention pipeline, f32 psum accumulation"):
        _emit(tc, xT, wA, wB, wpT, out)

    nc.compile()
    return nc


def _emit(tc, xT, wA, wB, wpT, out):
    nc = tc.nc
    mm = nc.tensor.matmul
    Exp = mybir.ActivationFunctionType.Exp
    MUL = mybir.AluOpType.mult

    from contextlib import ExitStack
    from itertools import chain

    with ExitStack() as ctx:
        sb = ctx.enter_context(tc.tile_pool(name="sb", bufs=1))
        rcp = ctx.enter_context(tc.tile_pool(name="rcp", bufs=2))
        finp = ctx.enter_context(tc.tile_pool(name="finp", bufs=6))
        pA = ctx.enter_context(tc.tile_pool(name="pA", bufs=3, space="PSUM"))
        pS = ctx.enter_context(tc.tile_pool(name="pS", bufs=2, space="PSUM"))

        def til(shape, dt, name):
            return sb.tile(shape, dt, name=name, tag=name)

        # ---- tiles ----
        wA_sb = til([P, 8, 512], BF16, "wA_sb")
        wB_sb = til([P, 8, 256], BF16, "wB_sb")
        wp_sb = til([P, 2, C], BF16, "wp_sb")
        xa = [til([P, NQ], BF16, f"xa{ci}") for ci in range(8)]
        xb = [til([P, NQ], BF16, f"xb{ci}") for ci in range(8)]
        qt = [til([P, NQ], BF16, f"qt{p}") for p in range(2)]
        kt = [[til([P, NQ], BF16, f"kt{p}_{hf}") for hf in range(2)] for p in range(2)]
        vb = [til([P, HPC, D + 1], BF16, f"vb{j}") for j in range(16)]
        # triple-buffered so attnv(h) may lag up to two heads behind exp(h)
        ets = [[til([P, NQ], BF16, f"ets{s}_{j}") for j in range(16)] for s in range(3)]
        ap_ = [til([P, 8, P], BF16, f"ap{p}") for p in range(2)]
        aT = [til([P, 8, P], BF16, f"aT{p}") for p in range(2)]
        ones = til([1, 1], BF16, "ones")
        onesD = til([1, D], BF16, "onesD")
        bc = til([D, NQ], BF16, "bc")

        # Pre-trigger the exp table load while DMAs stream.
        nc.vector.memset(ones, 1.0)
        nc.vector.memset(onesD, 1.0)
        dm = til([1, 1], BF16, "dm")
        nc.scalar.activation(out=dm, in_=ones, func=Exp, scale=1.0)


        # ---- loads: one HWDGE queue, exact consumption order ----
        wA_src = wA.rearrange("(a p) d -> p a d", p=P)
        for ci in range(8):
            nc.sync.dma_start(out=wA_sb[:, ci, :], in_=wA_src[:, ci, :])
            nc.sync.dma_start(out=xa[ci], in_=xT[ci * P:(ci + 1) * P, 0:NQ])
        for ci in range(8):
            nc.sync.dma_start(out=xb[ci], in_=xT[ci * P:(ci + 1) * P, NQ:N])
        nc.sync.dma_start(out=wB_sb, in_=wB.rearrange("(a p) d -> p a d", p=P))
        nc.sync.dma_start(out=wp_sb, in_=wpT.rearrange("(a p) d -> p a d", p=P))

        # ---- stage A: q/k pair-0 ci-outer + v pass 1 (ci 0..3), consuming
        # chunks on arrival (stage A is DMA-bound; the rest is B-phase filler)
        ps_q = pA.tile([P, NQ], F32, name="ps_q0", tag="pA")
        ps_k0 = pA.tile([P, NQ], F32, name="ps_k0_0", tag="pA")

        for ci in range(8):
            lw = wA_sb[:, ci, 0:P]
            for nh in range(2):
                mm(ps_q[:, nh * 512:(nh + 1) * 512], lw,
                   xa[ci][:, nh * 512:(nh + 1) * 512],
                   start=(ci == 0), stop=(ci == 7), skip_group_check=True)
            lw = wA_sb[:, ci, P:2 * P]
            for nh in range(2):
                mm(ps_k0[:, nh * 512:(nh + 1) * 512], lw,
                   xa[ci][:, nh * 512:(nh + 1) * 512],
                   start=(ci == 0), stop=(ci == 7), skip_group_check=True)
        # evacuations split across DVE and ACT (GPSIMD cannot read PSUM);
        # the exact regions scores(0,0..1) reads land first on each engine
        nc.vector.tensor_copy(qt[0][:, 0:512], ps_q[:, 0:512])
        nc.scalar.copy(qt[0][:, 512:1024], ps_q[:, 512:1024])
        nc.scalar.copy(kt[0][0][:, 0:256], ps_k0[:, 0:256])
        nc.vector.tensor_copy(kt[0][0][:, 256:1024], ps_k0[:, 256:1024])

        k0h1_ps = []

        def k0h1_gen():
            ps = pA.tile([P, NQ], F32, name="ps_k0_1", tag="pA")
            k0h1_ps.append(ps)
            for ci in range(8):
                lw = wA_sb[:, ci, P:2 * P]
                for nh in range(2):
                    mm(ps[:, nh * 512:(nh + 1) * 512], lw,
                       xb[ci][:, nh * 512:(nh + 1) * 512],
                       start=(ci == 0), stop=(ci == 7), skip_group_check=True)
                yield

        # ---- B-phase helpers ----
        def scores_j(h, j):
            pair, po = h // 2, D * (h % 2)
            ps = pA.tile([P, NQ], F32, name=f"ps_s{h}_{j}", tag="pA")
            hf, jj = j // 8, j % 8
            lw = kt[pair][hf][po:po + D, jj * P:(jj + 1) * P]
            for nh in range(2):
                mm(ps[:, nh * 512:(nh + 1) * 512], lw,
                   qt[pair][po:po + D, nh * 512:(nh + 1) * 512],
                   start=True, stop=True)
            nc.scalar.activation(out=ets[h % 3][j], in_=ps, func=Exp, scale=SCALE)

        def attnv_norm_m(h, m):
            ps = pS.tile([P, 512], F32, name=f"ps_av{h}_{m}", tag="pS")
            for j in range(16):
                mm(ps[:, 0:D + 1], ets[h % 3][j][:, m * P:(m + 1) * P],
                   vb[j][:, h, :], start=(j == 0), stop=(j == 15),
                   skip_group_check=True)
            p, os_ = h // 2, D * (h % 2)
            rc = rcp.tile([P, 1], F32, name=f"rc{h}_{m}", tag="rc")
            nc.vector.reciprocal(rc, ps[:, D:D + 1])
            nc.vector.tensor_scalar(ap_[p][:, m, os_:os_ + D], ps[:, 0:D],
                                    rc[:, 0:1], None, MUL)

        def proj_m(m):
            ps = pA.tile([P, NQ], F32, name=f"ps_f{m}", tag="pA")
            for p in range(2):
                lw = aT[p][:, m, :]
                for nh in range(2):
                    mm(ps[:, nh * 512:(nh + 1) * 512], lw,
                       wp_sb[:, p, nh * 512:(nh + 1) * 512],
                       start=(p == 0), stop=(p == 1), skip_group_check=True)
            fin = finp.tile([P, NQ], BF16, name=f"fin{m}", tag="fin")
            nc.scalar.copy(fin[:, 0:512], ps[:, 0:512])
            nc.vector.tensor_copy(fin[:, 512:1024], ps[:, 512:1024])
            eng = nc.scalar if m >= 6 else nc.sync
            eng.dma_start(out=out[m * P:(m + 1) * P, :], in_=fin)

        # ---- B-phase fillers: q/k pair-1, k pair-0 half-1 is already done;
        # v projection (all 8 chunks, one psum group per kv chunk)
        def q1_gen():
            ps = pA.tile([P, NQ], F32, name="ps_q1", tag="pA")
            for ci in range(8):
                lw = wB_sb[:, ci, 0:P]
                for nh in range(2):
                    mm(ps[:, nh * 512:(nh + 1) * 512], lw,
                       xa[ci][:, nh * 512:(nh + 1) * 512],
                       start=(ci == 0), stop=(ci == 7), skip_group_check=True)
                yield
            nc.vector.tensor_copy(qt[1], ps)
            yield

        k1h1_ps = []

        def k1_gen(hf):
            ps = pA.tile([P, NQ], F32, name=f"ps_k1_{hf}", tag="pA")
            if hf == 1:
                k1h1_ps.append(ps)
            xh = xa if hf == 0 else xb
            for ci in range(8):
                lw = wB_sb[:, ci, P:2 * P]
                for nh in range(2):
                    mm(ps[:, nh * 512:(nh + 1) * 512], lw,
                       xh[ci][:, nh * 512:(nh + 1) * 512],
                       start=(ci == 0), stop=(ci == 7), skip_group_check=True)
                yield
            if hf == 0:
                nc.vector.tensor_copy(kt[1][hf], ps)
                yield

        def v_gen():
            for j in range(16):
                ps = pS.tile([P, 512], F32, name=f"ps_v_{j}", tag="pS")
                nc.gpsimd.memset(vb[j][:, :, D:D + 1], 1.0)
                xh, jj = (xa, j) if j < 8 else (xb, j - 8)
                for ci in range(8):
                    mm(ps[:, 0:DH], xh[ci][:, jj * P:(jj + 1) * P], wA_sb[:, ci, 2 * P:],
                       start=(ci == 0), stop=(ci == 7), skip_group_check=True)
                    if ci % 4 == 3:
                        yield
                nc.vector.tensor_copy(
                    vb[j][:, :, 0:D],
                    ps[:, 0:DH].rearrange("p (h d) -> p h d", h=HPC))
                yield

        _SENT = object()

        def pull(gen, k):
            for _ in range(k):
                if next(gen, _SENT) is _SENT:
                    return False
            return True

        fillers = chain(k0h1_gen(), q1_gen(), k1_gen(0), v_gen(), k1_gen(1))

        # ---- B0: scores/exp head 0, fillers keep PE busy between slot waits
        # (the kt[0][1] evacuation is emitted immediately before its first
        # consumer: the scheduler's wait-coalescing loses the RAW edge when
        # other PE work sits between them)
        for j in range(16):
            if j == 8:
                nc.vector.tensor_copy(kt[0][1], k0h1_ps[0])
            scores_j(0, j)
            pull(fillers, 2)

        # ---- B1: scores head 1; fillers drain except the k1h1 tail, which
        # paces through B2 (kt[1][1] is first read at scores(3, 8), late B3)
        for j in range(16):
            scores_j(1, j)
            pull(fillers, 3)

        # head-3 half of ap_[1] is never written (head 3 skips the packed
        # path); zero it so its transposes move defined bits
        nc.gpsimd.memset(ap_[1][:, :, D:P], 0.0)

        # ---- B2: scores head 2 | attnv heads 0+1 | transposes pair 0 ----
        for m in range(8):
            if m == 0:
                # sync anchors: tiny matmuls touching qt[1]/kt[1][0]/vb force
                # the scheduler to order their producers before this point on
                # the PE queue (its wait-coalescing can otherwise lose RAW
                # edges to hoisted consumers further down)
                psa = pS.tile([P, 512], F32, name="ps_anchor", tag="pS")
                mm(psa[64:65, 500:501], qt[1][64:65, 0:1],
                   kt[1][0][64:65, 0:1], start=True, stop=True,
                   skip_group_check=True)
                mm(psa[64:65, 501:502], vb[15][64:65, 3, 0:1],
                   vb[14][64:65, 3, 0:1], start=True, stop=True,
                   skip_group_check=True)
            if m == 4:
                nc.vector.tensor_copy(kt[1][1], k1h1_ps[0])
            scores_j(2, 2 * m)
            scores_j(2, 2 * m + 1)
            pull(fillers, 1)
            attnv_norm_m(0, m)
            attnv_norm_m(1, m)
            eng = nc.sync if m % 2 == 0 else nc.scalar
            eng.dma_start_transpose(aT[0][:, m, :], ap_[0][:, m, :])
        for _ in fillers:
            pass

        # ---- head 3 runs v-STATIONARY (psum [65, q]): each exp chunk is
        # consumed as it is produced inside one open accumulation group, and
        # the output lands directly in [d, q] layout (aT rows) - no transpose
        # and no post-exp attnv stall for the final head. Head 2 stays on the
        # packed path; its transposes carry a zeroed head-3 half that the
        # head-3 normalization overwrites.
        ps_o3 = pA.tile([P, NQ], F32, name="ps_o3", tag="pA")

        def attnv_old3(j):
            lw = vb[j][:, 3, :]                  # [128, 65] (col 64 = ones)
            for nh in range(2):
                mm(ps_o3[0:D + 1, nh * 512:(nh + 1) * 512], lw,
                   ets[0][j][:, nh * 512:(nh + 1) * 512],
                   start=(j == 0), stop=(j == 15), skip_group_check=True)

        # ---- B3: scores head 3 | attnv head 3 chases its exp stream |
        # attnv head 2 + transposes pair 1 ----
        for m in range(8):
            scores_j(3, 2 * m)
            if m > 0:
                attnv_old3(2 * m - 2)
            scores_j(3, 2 * m + 1)
            if m > 0:
                attnv_old3(2 * m - 1)
            attnv_norm_m(2, m)
            eng = nc.sync if m % 2 == 0 else nc.scalar
            eng.dma_start_transpose(aT[1][:, m, :], ap_[1][:, m, :])
        attnv_old3(14)
        attnv_old3(15)

        # ---- B4 tail: normalize head 3 straight into aT rows, project ----
        # nh=0 chain completes first: proj(m<4) depends only on that half.
        # quarter-granular normalize chain: proj(0) only needs quarter 0,
        # so it starts ~2us after the last exp instead of ~3.5
        rc = rcp.tile([1, NQ], BF16, name="rco3", tag="rco")
        aTv = aT[1][D:2 * D, :, :].rearrange("p a b -> p (a b)")
        qsl = [slice(qq * 256, (qq + 1) * 256) for qq in range(4)]
        pbs = []
        # interleaved per-quarter chains: recip -> pb-mm -> ACT copy -> mul,
        # DVE order r0 r1 m0 r2 m1 r3 m2 m3 so proj(0) unblocks ~1us sooner
        for qq in range(4):
            nc.vector.reciprocal(rc[0:1, qsl[qq]], ps_o3[D:D + 1, qsl[qq]])
            if qq % 2 == 0:
                pbs.append(pS.tile([P, 512], F32, name=f"pb3_{qq}", tag="pS"))
            mm(pbs[qq // 2][0:D, (qq % 2) * 256:(qq % 2) * 256 + 256], onesD,
               rc[0:1, qsl[qq]], start=True, stop=True)
            nc.scalar.copy(bc[:, qsl[qq]], pbs[qq // 2][0:D, qsl[qq % 2]])
            if qq >= 1:
                q0 = qq - 1
                nc.vector.tensor_mul(aTv[:, qsl[q0]], ps_o3[0:D, qsl[q0]],
                                     bc[:, qsl[q0]])
        nc.vector.tensor_mul(aTv[:, qsl[3]], ps_o3[0:D, qsl[3]], bc[:, qsl[3]])
        for m in range(8):
            proj_m(m)


def _get_nc():
    if "nc" not in _CACHE:
        _CACHE["nc"] = _build()
    return _CACHE["nc"]


def kernel(x, wq, wk, wv, w_proj, b_proj):
    x = np.asarray(x, dtype=np.float32)
    wq = np.asarray(wq, dtype=np.float32)
    wk = np.asarray(wk, dtype=np.float32)
    wv = np.asarray(wv, dtype=np.float32)
    w_proj = np.asarray(w_proj, dtype=np.float32)
    b_proj = np.asarray(b_proj, dtype=np.float32)
    bf = ml_dtypes.bfloat16

    nc = _get_nc()
    in_maps = []
    for core in range(8):
        b, g = divmod(core, 4)
        sl = slice(g * DH, (g + 1) * DH)
        wqT = wq[sl, :].T
        wkT = wk[sl, :].T
        wvT = wv[sl, :].T
        in_maps.append({
            "xT": np.ascontiguousarray(x[b].T).astype(bf),
            "wA": np.ascontiguousarray(
                np.hstack([wqT[:, 0:P], wkT[:, 0:P], wvT])).astype(bf),
            "wB": np.ascontiguousarray(
                np.hstack([wqT[:, P:2 * P], wkT[:, P:2 * P]])).astype(bf),
            "wpT": np.ascontiguousarray(w_proj[:, sl].T).astype(bf),
        })

    res = run_bass_kernel_spmd(nc, in_maps, core_ids=list(range(8)),
                               trace=bool(int(os.environ.get("KERNEL_TRACE", "0"))))
    _CACHE["last_results"] = res
    outs = [np.asarray(res.results[c]["out"]).astype(np.float32) for c in range(8)]
    full = np.stack([outs[0] + outs[1] + outs[2] + outs[3],
                     outs[4] + outs[5] + outs[6] + outs[7]])
    full += b_proj[None, None, :]
    return full.astype(np.float32)
